# revision 2
# baseline (speedup 1.0000x reference)
"""VQ codebook (KMeans) kernel for 8 Trainium2 NeuronCores.

Strategy (data-parallel per sharding hint):
  - shard the flattened N=65536 rows of x across 8 cores (8192 rows each),
    replicate the 1024x1024 codebook.
  - per core: scores S = x @ C^T - 0.5*||c_k||^2 computed on the PE with a
    bf16 hi/lo 3-pass decomposition (xh*ch + xh*cl + xl*ch), which preserves
    fp32-grade argmax fidelity at 3 bf16-matmul passes instead of a 4x-cost
    fp32 matmul.
  - argmax over K=1024 via the DVE max/max_index ops (first-index tie-break,
    matching jnp.argmax).
  - quantize rows gathered from the fp32 codebook in DRAM via indirect DMA.
  - codebook_loss = (quantize - x)^2 elementwise on the DVE in fp32.
  - per-row indices are returned to the host; the host does the (tiny)
    bincount + perplexity reduction in fp32, mirroring the reference.

The matmul needs x with the d-axis on partitions, so the host ships both the
natural fp32 shard (for the loss) and hi/lo bf16 transposed shards (matmul
stationary operands). Total device traffic/core: ~164 MB; PE work: 48
N=512 bf16 matmuls per 128-row tile.
"""

import sys

if "/opt/trn_rl_repo" not in sys.path:
    sys.path.insert(0, "/opt/trn_rl_repo")

import numpy as np
import ml_dtypes

import concourse.bacc as bacc
import concourse.bass as bass
import concourse.mybir as mybir
from concourse.tile import TileContext
from concourse.bass_utils import run_bass_kernel_spmd

BF16 = ml_dtypes.bfloat16
P = 128
D = 1024
K = 1024
KH = 512
D_CHUNKS = D // P  # 8
N_CORES = 8
FULL_SHAPE = (16, 4096, D)
N_ROWS = FULL_SHAPE[0] * FULL_SHAPE[1]  # 65536
ROWS_PER_CORE = N_ROWS // N_CORES  # 8192
BLK_TILES = 4  # row-tiles per transposed-x staging block

_gen_cache = {}


def _gen(n_tiles: int):
    rows = n_tiles * P
    nc = bacc.Bacc("TRN2", target_bir_lowering=False, debug=False)
    x_nat = nc.dram_tensor("x_nat", [rows, D], mybir.dt.float32, kind="ExternalInput")
    xt_h = nc.dram_tensor("xt_h", [D, rows], mybir.dt.bfloat16, kind="ExternalInput")
    xt_l = nc.dram_tensor("xt_l", [D, rows], mybir.dt.bfloat16, kind="ExternalInput")
    cb = nc.dram_tensor("cb", [K, D], mybir.dt.float32, kind="ExternalInput")
    ct_h = nc.dram_tensor("ct_h", [D, K], mybir.dt.bfloat16, kind="ExternalInput")
    ct_l = nc.dram_tensor("ct_l", [D, K], mybir.dt.bfloat16, kind="ExternalInput")
    q_out = nc.dram_tensor("q_out", [rows, D], mybir.dt.float32, kind="ExternalOutput")
    l_out = nc.dram_tensor("l_out", [rows, D], mybir.dt.float32, kind="ExternalOutput")
    i_out = nc.dram_tensor("i_out", [rows, 1], mybir.dt.uint32, kind="ExternalOutput")

    with TileContext(nc) as tc:
        with (
            tc.tile_pool(name="const", bufs=1) as cpool,
            tc.tile_pool(name="xt", bufs=2) as xtpool,
            tc.tile_pool(name="work", bufs=3) as wpool,
            tc.tile_pool(name="psum", bufs=2, space="PSUM") as pspool,
            tc.tile_pool(name="pre_ps", bufs=1, space="PSUM") as prepool,
        ):
            # Codebook^T hi/lo resident in SBUF: [128, c*K + k] = C^T[c*128+p, k]
            cth = cpool.tile([P, D_CHUNKS * K], mybir.dt.bfloat16)
            ctl = cpool.tile([P, D_CHUNKS * K], mybir.dt.bfloat16)
            for c in range(D_CHUNKS):
                nc.sync.dma_start(cth[:, c * K:(c + 1) * K], ct_h[c * P:(c + 1) * P, :])
                nc.sync.dma_start(ctl[:, c * K:(c + 1) * K], ct_l[c * P:(c + 1) * P, :])

            # bias[k] = -0.5 * sum_d C[k,d]^2, materialized broadcast across
            # partitions via an all-ones stationary matmul over (ch+cl)^2.
            ones = cpool.tile([P, P], mybir.dt.float32)
            nc.gpsimd.memset(ones[:], 1.0)
            bias_ps = prepool.tile([P, K], mybir.dt.float32)
            for c in range(D_CHUNKS):
                csum = wpool.tile([P, K], mybir.dt.float32, tag="csum")
                nc.vector.tensor_tensor(
                    out=csum[:],
                    in0=cth[:, c * K:(c + 1) * K],
                    in1=ctl[:, c * K:(c + 1) * K],
                    op=mybir.AluOpType.add,
                )
                csq = wpool.tile([P, K], mybir.dt.float32, tag="csq")
                nc.vector.tensor_tensor(
                    out=csq[:], in0=csum[:], in1=csum[:], op=mybir.AluOpType.mult
                )
                for h in range(2):
                    nc.tensor.matmul(
                        bias_ps[:, h * KH:(h + 1) * KH],
                        ones[:],
                        csq[:, h * KH:(h + 1) * KH],
                        start=(c == 0),
                        stop=(c == D_CHUNKS - 1),
                    )
            bias_sb = cpool.tile([P, K], mybir.dt.float32)
            nc.vector.tensor_scalar_mul(bias_sb[:], bias_ps[:], -0.5)

            for b in range((n_tiles + BLK_TILES - 1) // BLK_TILES):
                t0 = b * BLK_TILES
                bt = min(BLK_TILES, n_tiles - t0)
                bw = bt * P
                xh_blk = xtpool.tile([P, D_CHUNKS * bw], mybir.dt.bfloat16, tag="xh")
                xl_blk = xtpool.tile([P, D_CHUNKS * bw], mybir.dt.bfloat16, tag="xl")
                for c in range(D_CHUNKS):
                    nc.sync.dma_start(
                        xh_blk[:, c * bw:(c + 1) * bw],
                        xt_h[c * P:(c + 1) * P, t0 * P:t0 * P + bw],
                    )
                    nc.sync.dma_start(
                        xl_blk[:, c * bw:(c + 1) * bw],
                        xt_l[c * P:(c + 1) * P, t0 * P:t0 * P + bw],
                    )
                for ti in range(bt):
                    t = t0 + ti
                    r0 = t * P
                    xtile = wpool.tile([P, D], mybir.dt.float32, tag="xnat")
                    nc.sync.dma_start(xtile[:], x_nat[r0:r0 + P, :])

                    ps = pspool.tile([P, K], mybir.dt.float32, tag="score")
                    for c in range(D_CHUNKS):
                        wh = xh_blk[:, c * bw + ti * P:c * bw + (ti + 1) * P]
                        wl = xl_blk[:, c * bw + ti * P:c * bw + (ti + 1) * P]
                        # group by stationary operand: 4 MMs on xh, 2 on xl
                        for w, cts, is_last_grp in (
                            (wh, (cth, ctl), False),
                            (wl, (cth,), True),
                        ):
                            for ct_sb in cts:
                                for h in range(2):
                                    nc.tensor.matmul(
                                        ps[:, h * KH:(h + 1) * KH],
                                        w,
                                        ct_sb[:, c * K + h * KH:c * K + (h + 1) * KH],
                                        start=(c == 0 and w is wh and ct_sb is cth),
                                        stop=(
                                            c == D_CHUNKS - 1
                                            and is_last_grp
                                            and ct_sb is cts[-1]
                                        ),
                                    )

                    sc = wpool.tile([P, K], mybir.dt.float32, tag="sc")
                    nc.vector.tensor_tensor(
                        out=sc[:], in0=ps[:], in1=bias_sb[:], op=mybir.AluOpType.add
                    )
                    mx8 = wpool.tile([P, 8], mybir.dt.float32, tag="mx8")
                    idx8 = wpool.tile([P, 8], mybir.dt.uint32, tag="idx8")
                    nc.vector.max(mx8[:], sc[:])
                    nc.vector.max_index(idx8[:], mx8[:], sc[:])

                    qt = wpool.tile([P, D], mybir.dt.float32, tag="q")
                    nc.gpsimd.indirect_dma_start(
                        out=qt[:],
                        out_offset=None,
                        in_=cb[:, :],
                        in_offset=bass.IndirectOffsetOnAxis(ap=idx8[:, :1], axis=0),
                    )
                    nc.sync.dma_start(q_out[r0:r0 + P, :], qt[:])
                    nc.sync.dma_start(i_out[r0:r0 + P, :], idx8[:, :1])

                    df = wpool.tile([P, D], mybir.dt.float32, tag="df")
                    nc.vector.tensor_tensor(
                        out=df[:], in0=qt[:], in1=xtile[:], op=mybir.AluOpType.subtract
                    )
                    ls = wpool.tile([P, D], mybir.dt.float32, tag="ls")
                    nc.vector.tensor_tensor(
                        out=ls[:], in0=df[:], in1=df[:], op=mybir.AluOpType.mult
                    )
                    nc.sync.dma_start(l_out[r0:r0 + P, :], ls[:])
    nc.compile()
    return nc


def _get_nc(n_tiles: int):
    if n_tiles not in _gen_cache:
        _gen_cache[n_tiles] = _gen(n_tiles)
    return _gen_cache[n_tiles]


def _prep_inputs(x: np.ndarray, codebook: np.ndarray, n_cores: int, rows_per_core: int):
    f = np.ascontiguousarray(x.reshape(-1, D), dtype=np.float32)
    ct = np.ascontiguousarray(codebook.T.astype(np.float32))
    ct_h = ct.astype(BF16)
    ct_l = (ct - ct_h.astype(np.float32)).astype(BF16)
    xh = f.astype(BF16)
    xl = (f - xh.astype(np.float32)).astype(BF16)
    in_maps = []
    for c in range(n_cores):
        sl = slice(c * rows_per_core, (c + 1) * rows_per_core)
        in_maps.append(
            {
                "x_nat": f[sl],
                "xt_h": np.ascontiguousarray(xh[sl].T),
                "xt_l": np.ascontiguousarray(xl[sl].T),
                "cb": codebook.astype(np.float32),
                "ct_h": ct_h,
                "ct_l": ct_l,
            }
        )
    return in_maps


def run_sharded(x: np.ndarray, codebook: np.ndarray, trace: bool = False):
    """Run the SPMD kernel; returns (quantize, loss, perp, indices, results)."""
    in_maps = _prep_inputs(x, codebook, N_CORES, ROWS_PER_CORE)
    nc = _get_nc(ROWS_PER_CORE // P)
    try:
        res = run_bass_kernel_spmd(
            nc, in_maps, core_ids=list(range(N_CORES)), trace=trace
        )
    except ModuleNotFoundError:
        # NTFF profiling hook unavailable in this client; run without trace.
        res = run_bass_kernel_spmd(
            nc, in_maps, core_ids=list(range(N_CORES)), trace=False
        )
    q = np.concatenate([r["q_out"] for r in res.results], axis=0)
    ls = np.concatenate([r["l_out"] for r in res.results], axis=0)
    ind = np.concatenate([r["i_out"] for r in res.results], axis=0).ravel()
    counts = np.bincount(ind, minlength=K).astype(np.float32)
    prob = counts / np.float32(ind.shape[0])
    perp = np.exp(
        -np.sum(prob * np.log(prob + np.float32(1e-10)), dtype=np.float32)
    ).astype(np.float32)
    quantize = q.reshape(FULL_SHAPE)
    loss = ls.reshape(FULL_SHAPE)
    return quantize, loss, perp, ind, res


def kernel(x: np.ndarray, codebook: np.ndarray):
    quantize, loss, perp, _, _ = run_sharded(x, codebook, trace=False)
    return quantize, loss, perp


if __name__ == "__main__":
    rng = np.random.default_rng(0)
    x = rng.standard_normal((16, 4096, D), dtype=np.float32)
    cbk = rng.standard_normal((K, D), dtype=np.float32)
    q, ls, perp = kernel(x, cbk)
    print(q.shape, ls.shape, perp)


# revision 6
# speedup vs baseline: 2.8284x; 2.8284x over previous
"""VQ codebook (KMeans) kernel for 8 Trainium2 NeuronCores.

Strategy (data-parallel per sharding hint):
  - shard the flattened N=65536 rows of x across 8 cores (8192 rows each),
    replicate the 1024x1024 codebook.
  - per core: scores S = x @ C^T - 0.5*||c_k||^2 computed on the PE with a
    bf16 hi/lo 3-pass decomposition (xh*ch + xh*cl + xl*ch), which preserves
    fp32-grade argmax fidelity at 3 bf16-matmul passes instead of a 4x-cost
    fp32 matmul.
  - argmax over K=1024 via the DVE max/max_index ops (first-index tie-break,
    matching jnp.argmax).
  - quantize rows gathered from the fp32 codebook in DRAM via indirect DMA
    (SWDGE ring, overlaps the HWDGE streams).
  - codebook_loss = (quantize - x)^2 elementwise on the DVE in fp32.
  - per-row indices are returned to the host; the host does the (tiny)
    bincount + perplexity reduction in fp32, mirroring the reference.

IO is batched at 512-row supertiles (2 MB per DMA) and split across the two
HWDGE rings (inputs on sync, outputs on scalar) because HWDGE transfers
serialize per ring at ~2us fixed cost each.
"""

import sys

if "/opt/trn_rl_repo" not in sys.path:
    sys.path.insert(0, "/opt/trn_rl_repo")

import numpy as np
import ml_dtypes

import concourse.bacc as bacc
import concourse.bass as bass
import concourse.mybir as mybir
from concourse.tile import TileContext
from concourse.bass_utils import run_bass_kernel_spmd

BF16 = ml_dtypes.bfloat16
P = 128
D = 1024
K = 1024
KH = 512
D_CHUNKS = D // P  # 8
N_CORES = 8
FULL_SHAPE = (16, 4096, D)
N_ROWS = FULL_SHAPE[0] * FULL_SHAPE[1]  # 65536
ROWS_PER_CORE = N_ROWS // N_CORES  # 8192
SUP = 4  # row-tiles per supertile (512 rows)

_gen_cache = {}


def _gen(n_tiles: int, reps: int = 1):
    rows = n_tiles * P
    nc = bacc.Bacc("TRN2", target_bir_lowering=False, debug=False)
    x_nat = nc.dram_tensor("x_nat", [rows, D], mybir.dt.float32, kind="ExternalInput")
    xt_h = nc.dram_tensor("xt_h", [D, rows], mybir.dt.bfloat16, kind="ExternalInput")
    xt_l = nc.dram_tensor("xt_l", [D, rows], mybir.dt.bfloat16, kind="ExternalInput")
    cb = nc.dram_tensor("cb", [K, D], mybir.dt.float32, kind="ExternalInput")
    ct_h = nc.dram_tensor("ct_h", [D, K], mybir.dt.bfloat16, kind="ExternalInput")
    ct_l = nc.dram_tensor("ct_l", [D, K], mybir.dt.bfloat16, kind="ExternalInput")
    q_out = nc.dram_tensor("q_out", [rows, D], mybir.dt.float32, kind="ExternalOutput")
    l_out = nc.dram_tensor("l_out", [rows, D], mybir.dt.float32, kind="ExternalOutput")
    i_out = nc.dram_tensor("i_out", [rows, 1], mybir.dt.uint32, kind="ExternalOutput")

    n_sup = (n_tiles + SUP - 1) // SUP

    with TileContext(nc) as tc:
        with (
            tc.tile_pool(name="const", bufs=1) as cpool,
            tc.tile_pool(name="xt", bufs=2) as xtpool,
            tc.tile_pool(name="blk", bufs=2) as bpool,
            tc.tile_pool(name="work", bufs=3) as wpool,
            tc.tile_pool(name="psum", bufs=3, space="PSUM") as pspool,
            tc.tile_pool(name="pre_ps", bufs=1, space="PSUM") as prepool,
        ):
            # Codebook^T hi/lo resident in SBUF: [128, c*K + k] = C^T[c*128+p, k]
            cth = cpool.tile([P, D_CHUNKS * K], mybir.dt.bfloat16)
            ctl = cpool.tile([P, D_CHUNKS * K], mybir.dt.bfloat16)
            nc.sync.dma_start(
                cth[:].rearrange("p (c k) -> p c k", c=D_CHUNKS),
                ct_h[:, :].rearrange("(c p) k -> p c k", p=P),
            )
            nc.sync.dma_start(
                ctl[:].rearrange("p (c k) -> p c k", c=D_CHUNKS),
                ct_l[:, :].rearrange("(c p) k -> p c k", p=P),
            )

            # bias[k] = -0.5 * sum_d C[k,d]^2, materialized broadcast across
            # partitions via an all-ones stationary matmul over (ch+cl)^2.
            ones = cpool.tile([P, P], mybir.dt.float32)
            nc.gpsimd.memset(ones[:], 1.0)
            bias_ps = prepool.tile([P, K], mybir.dt.float32)
            for c in range(D_CHUNKS):
                csum = wpool.tile([P, K], mybir.dt.float32, tag="csum")
                nc.vector.tensor_tensor(
                    out=csum[:],
                    in0=cth[:, c * K:(c + 1) * K],
                    in1=ctl[:, c * K:(c + 1) * K],
                    op=mybir.AluOpType.add,
                )
                csq = wpool.tile([P, K], mybir.dt.float32, tag="csq")
                nc.vector.tensor_tensor(
                    out=csq[:], in0=csum[:], in1=csum[:], op=mybir.AluOpType.mult
                )
                for h in range(2):
                    nc.tensor.matmul(
                        bias_ps[:, h * KH:(h + 1) * KH],
                        ones[:],
                        csq[:, h * KH:(h + 1) * KH],
                        start=(c == 0),
                        stop=(c == D_CHUNKS - 1),
                    )
            bias_sb = cpool.tile([P, K], mybir.dt.float32)
            nc.vector.tensor_scalar_mul(bias_sb[:], bias_ps[:], -0.5)

            def body():
              for s in range(n_sup):
                t0 = s * SUP
                st = min(SUP, n_tiles - t0)
                bw = st * P  # rows in this supertile
                r0 = t0 * P
                # transposed hi/lo x for the matmuls (sync ring, 1 DMA each)
                xh_blk = xtpool.tile([P, D_CHUNKS * bw], mybir.dt.bfloat16, tag="xh")
                xl_blk = xtpool.tile([P, D_CHUNKS * bw], mybir.dt.bfloat16, tag="xl")
                nc.sync.dma_start(
                    xh_blk[:].rearrange("p (c w) -> p c w", c=D_CHUNKS),
                    xt_h[:, r0:r0 + bw].rearrange("(c p) w -> p c w", p=P),
                )
                nc.sync.dma_start(
                    xl_blk[:].rearrange("p (c w) -> p c w", c=D_CHUNKS),
                    xt_l[:, r0:r0 + bw].rearrange("(c p) w -> p c w", p=P),
                )
                # natural x for the loss (sync ring, 1 DMA)
                x_blk = bpool.tile([P, st * D], mybir.dt.float32, tag="x")
                nc.sync.dma_start(
                    x_blk[:].rearrange("p (j d) -> p j d", j=st),
                    x_nat[r0:r0 + bw, :].rearrange("(j p) d -> p j d", p=P),
                )

                q_blk = bpool.tile([P, st * D], mybir.dt.float32, tag="q")
                idx_blk = wpool.tile([P, SUP], mybir.dt.uint32, tag="idxb")

                for j in range(st):
                    ps = pspool.tile([P, K], mybir.dt.float32, tag="score")
                    for c in range(D_CHUNKS):
                        wh = xh_blk[:, c * bw + j * P:c * bw + (j + 1) * P]
                        wl = xl_blk[:, c * bw + j * P:c * bw + (j + 1) * P]
                        # group by stationary operand: 4 MMs on xh, 2 on xl
                        for w, cts, is_last_grp in (
                            (wh, (cth, ctl), False),
                            (wl, (cth,), True),
                        ):
                            for ct_sb in cts:
                                for h in range(2):
                                    nc.tensor.matmul(
                                        ps[:, h * KH:(h + 1) * KH],
                                        w,
                                        ct_sb[:, c * K + h * KH:c * K + (h + 1) * KH],
                                        start=(c == 0 and w is wh and ct_sb is cth),
                                        stop=(
                                            c == D_CHUNKS - 1
                                            and is_last_grp
                                            and ct_sb is cts[-1]
                                        ),
                                    )

                    sc = wpool.tile([P, K], mybir.dt.float32, tag="sc")
                    nc.vector.tensor_tensor(
                        out=sc[:], in0=ps[:], in1=bias_sb[:], op=mybir.AluOpType.add
                    )
                    mx8 = wpool.tile([P, 8], mybir.dt.float32, tag="mx8")
                    idx8 = wpool.tile([P, 8], mybir.dt.uint32, tag="idx8")
                    nc.vector.max(mx8[:], sc[:])
                    nc.vector.max_index(idx8[:], mx8[:], sc[:])
                    nc.vector.tensor_copy(idx_blk[:, j:j + 1], idx8[:, :1])
                    # per-tile indirect gather into this supertile's q block
                    nc.gpsimd.indirect_dma_start(
                        out=q_blk[:, j * D:(j + 1) * D],
                        out_offset=None,
                        in_=cb[:, :],
                        in_offset=bass.IndirectOffsetOnAxis(ap=idx8[:, :1], axis=0),
                    )

                # loss on the whole supertile
                ls_blk = bpool.tile([P, st * D], mybir.dt.float32, tag="ls")
                nc.vector.tensor_tensor(
                    out=ls_blk[:], in0=q_blk[:], in1=x_blk[:],
                    op=mybir.AluOpType.subtract,
                )
                nc.vector.tensor_tensor(
                    out=ls_blk[:], in0=ls_blk[:], in1=ls_blk[:],
                    op=mybir.AluOpType.mult,
                )
                # outputs on the scalar HWDGE ring
                nc.scalar.dma_start(
                    q_out[r0:r0 + bw, :].rearrange("(j p) d -> p j d", p=P),
                    q_blk[:].rearrange("p (j d) -> p j d", j=st),
                )
                nc.scalar.dma_start(
                    l_out[r0:r0 + bw, :].rearrange("(j p) d -> p j d", p=P),
                    ls_blk[:].rearrange("p (j d) -> p j d", j=st),
                )
                nc.scalar.dma_start(
                    i_out[r0:r0 + bw, :].rearrange("(j p) o -> p j o", p=P),
                    idx_blk[:, :st].rearrange("p (j o) -> p j o", o=1),
                )

            if reps > 1:
                with tc.For_i(0, reps, 1):
                    body()
            else:
                body()
    nc.compile()
    return nc


def _get_nc(n_tiles: int):
    if n_tiles not in _gen_cache:
        _gen_cache[n_tiles] = _gen(n_tiles)
    return _gen_cache[n_tiles]


def _prep_inputs(x: np.ndarray, codebook: np.ndarray, n_cores: int, rows_per_core: int):
    f = np.ascontiguousarray(x.reshape(-1, D), dtype=np.float32)
    ct = np.ascontiguousarray(codebook.T.astype(np.float32))
    ct_h = ct.astype(BF16)
    ct_l = (ct - ct_h.astype(np.float32)).astype(BF16)
    xh = f.astype(BF16)
    xl = (f - xh.astype(np.float32)).astype(BF16)
    in_maps = []
    for c in range(n_cores):
        sl = slice(c * rows_per_core, (c + 1) * rows_per_core)
        in_maps.append(
            {
                "x_nat": f[sl],
                "xt_h": np.ascontiguousarray(xh[sl].T),
                "xt_l": np.ascontiguousarray(xl[sl].T),
                "cb": codebook.astype(np.float32),
                "ct_h": ct_h,
                "ct_l": ct_l,
            }
        )
    return in_maps


def run_sharded(x: np.ndarray, codebook: np.ndarray, trace: bool = False):
    """Run the SPMD kernel; returns (quantize, loss, perp, indices, results)."""
    in_maps = _prep_inputs(x, codebook, N_CORES, ROWS_PER_CORE)
    nc = _get_nc(ROWS_PER_CORE // P)
    try:
        res = run_bass_kernel_spmd(
            nc, in_maps, core_ids=list(range(N_CORES)), trace=trace
        )
    except ModuleNotFoundError:
        # NTFF profiling hook unavailable in this client; run without trace.
        res = run_bass_kernel_spmd(
            nc, in_maps, core_ids=list(range(N_CORES)), trace=False
        )
    q = np.concatenate([r["q_out"] for r in res.results], axis=0)
    ls = np.concatenate([r["l_out"] for r in res.results], axis=0)
    ind = np.concatenate([r["i_out"] for r in res.results], axis=0).ravel()
    counts = np.bincount(ind, minlength=K).astype(np.float32)
    prob = counts / np.float32(ind.shape[0])
    perp = np.exp(
        -np.sum(prob * np.log(prob + np.float32(1e-10)), dtype=np.float32)
    ).astype(np.float32)
    quantize = q.reshape(FULL_SHAPE)
    loss = ls.reshape(FULL_SHAPE)
    return quantize, loss, perp, ind, res


def kernel(x: np.ndarray, codebook: np.ndarray):
    quantize, loss, perp, _, _ = run_sharded(x, codebook, trace=False)
    return quantize, loss, perp


if __name__ == "__main__":
    rng = np.random.default_rng(0)
    x = rng.standard_normal((16, 4096, D), dtype=np.float32)
    cbk = rng.standard_normal((K, D), dtype=np.float32)
    q, ls, perp = kernel(x, cbk)
    print(q.shape, ls.shape, perp)
